# revision 12
# baseline (speedup 1.0000x reference)
"""HGTNet Trainium2 kernel: 8-core SPMD, destination-ownership edge sharding.

v2 strategy:
  - AllGather RAW hidden rows (bf16, Shared scratchpad) once per layer
    (h_tab [200000,128] = 51MB); k/v computed per edge-group on the tensor
    engine from gathered h (relation-pure groups of 128 edge slots).
  - k-bias folded into 8 augmented q-table columns (q_aug = h@Wq_aug);
    v-bias applied via per-relation softmax-mass (z_w/z_c) outer fixup.
    No per-group bias matmuls.
  - onehot from resident drl columns + is_equal; onehotT from a broadcast
    DMA of drl rows + is_equal (DMA engines are otherwise idle).
  - bf16 edge path; fp32 PSUM accumulation; fp32 out-phase mix/ELU chain
    (bf16 there was the dominant error term: 4e-2 -> 4e-3).
  - q tables and aggT SBUF-resident; idx/drl tables SBUF-resident.
  - Layer L-1 skips the rev pass / q_a / out_a entirely (h_a is dead);
    classifier fused into the last out_p pass.
"""
import sys
sys.path.insert(0, '/opt/trn_rl_repo')
import math
import numpy as np
import ml_dtypes

import concourse.bass as bass
import concourse.bacc as bacc
import concourse.mybir as mybir
import concourse.tile as tile
from concourse.bass_utils import run_bass_kernel_spmd

P = 128
D = 128
H = 4
DH = 32
L = 2
OUT = 40
C = 8
QW = D + 2 * H          # augmented q width (136)
SCALE = 1.0 / math.sqrt(DH)

F32 = mybir.dt.float32
BF16 = mybir.dt.bfloat16
I32 = mybir.dt.int32
BF = ml_dtypes.bfloat16

USE_SHARED = True
AF = mybir.ActivationFunctionType


def _ceil_div(a, b):
    return (a + b - 1) // b


# ---------------------------------------------------------------- host prep
def _bucket(src_rows, dst, nl, nb):
    core = dst // nl
    dl = dst % nl
    blk = dl // P
    drl = (dl % P).astype(np.float32)
    key = core.astype(np.int64) * nb + blk
    order = np.argsort(key, kind='stable')
    skey = key[order]
    counts = np.bincount(skey, minlength=C * nb)
    T = int(_ceil_div(int(counts.max()), P))
    offs = np.zeros(C * nb + 1, np.int64)
    offs[1:] = np.cumsum(counts)
    pos = np.arange(len(order), dtype=np.int64) - offs[skey]

    idx_flat = np.zeros((C * nb, T * P), np.int32)
    drl_flat = np.full((C * nb, T * P), -1.0, np.float32)
    idx_flat[skey, pos] = src_rows[order].astype(np.int32)
    drl_flat[skey, pos] = drl[order]
    return (idx_flat.reshape(C, nb, T, P), drl_flat.reshape(C, nb, T, P), T)


def _blockdiag(mats):
    out = np.zeros((D, D), np.float32)
    for h in range(H):
        out[h * DH:(h + 1) * DH, h * DH:(h + 1) * DH] = mats[h]
    return out


def host_prep(inputs, nl):
    nb = _ceil_div(nl, P)
    x_a = np.asarray(inputs['x_author'], np.float32)
    x_p = np.asarray(inputs['x_paper'], np.float32)
    ei_w = np.asarray(inputs['ei_writes']).astype(np.int64)
    ei_r = np.asarray(inputs['ei_rev']).astype(np.int64)
    ei_c = np.asarray(inputs['ei_cites']).astype(np.int64)
    kqv_w = np.asarray(inputs['kqv_w'], np.float32)
    kqv_b = np.asarray(inputs['kqv_b'], np.float32)
    rel_a = np.asarray(inputs['rel_a'], np.float32)
    rel_m = np.asarray(inputs['rel_m'], np.float32)
    p_rel = np.asarray(inputs['p_rel'], np.float32)

    idx_w, drl_w, TW = _bucket(ei_w[0], ei_w[1], nl, nb)
    idx_c, drl_c, TC = _bucket(ei_c[0], ei_c[1], nl, nb)
    idx_r, drl_r, TR = _bucket(ei_r[0], ei_r[1], nl, nb)
    TP = TW + TC

    def res_col(a):
        return np.ascontiguousarray(a.transpose(0, 3, 1, 2).reshape(C, P, -1))

    idx_pap = np.concatenate([idx_c, idx_w], axis=2)   # [C, nb, TP, P]
    drl_pap = np.concatenate([drl_c, drl_w], axis=2)
    idxP = res_col(idx_pap)
    idxR = res_col(idx_r)
    drlP = res_col(drl_pap).astype(BF)
    drlR = res_col(drl_r).astype(BF)
    drlrP = np.ascontiguousarray(drl_pap.reshape(C, nb, TP * P).astype(BF))
    drlrR = np.ascontiguousarray(drl_r.reshape(C, nb, TR * P).astype(BF))

    # folded per-relation kv weights (NO bias) packed [D, 768]: w, c, r
    rel_cfg = {'w': (0, 0), 'c': (1, 2), 'r': (1, 1)}
    Wkv3 = np.zeros((L, D, 3 * 256), np.float32)
    bk, bv = {}, {}
    for l in range(L):
        for j, nm in enumerate(('w', 'c', 'r')):
            t, r = rel_cfg[nm]
            Ra = _blockdiag(rel_a[l, r] * p_rel[l, r][:, None, None] * SCALE)
            Rm = _blockdiag(rel_m[l, r])
            Wkv3[l, :, j * 256:j * 256 + 128] = kqv_w[l, 0, t] @ Ra
            Wkv3[l, :, j * 256 + 128:(j + 1) * 256] = kqv_w[l, 2, t] @ Rm
            bk[(l, nm)] = kqv_b[l, 0, t] @ Ra
            bv[(l, nm)] = kqv_b[l, 2, t] @ Rm
    Wkv3 = Wkv3.astype(BF)

    # augmented q: cols 0:128 q; 128:132 q.bk_rel0 per head; 132:136 rel1.
    # dst type 0 (authors): rel0 = 'r'. dst type 1 (papers): rel0='w', rel1='c'.
    Wq_aug = np.zeros((L, 2, D, QW), np.float32)
    Bq_aug = np.zeros((L, 2, 1, QW), np.float32)
    for l in range(L):
        for t in (0, 1):
            Wq = kqv_w[l, 1, t]
            bq = kqv_b[l, 1, t]
            Wq_aug[l, t, :, :D] = Wq
            Bq_aug[l, t, 0, :D] = bq
            rels = ('r',) if t == 0 else ('c', 'w')
            for j, nm in enumerate(rels):
                bkr = bk[(l, nm)]
                for h in range(H):
                    blk = slice(h * DH, (h + 1) * DH)
                    Wq_aug[l, t, :, D + j * H + h] = Wq[:, blk] @ bkr[blk]
                    Bq_aug[l, t, 0, D + j * H + h] = bq[blk] @ bkr[blk]
    Wq_aug = Wq_aug.astype(BF)
    Bq_rep = np.broadcast_to(Bq_aug, (L, 2, P, QW)).astype(np.float32).copy()

    # v-bias fixup tiles [L, 3, P, D] f32 replicated rows: w, c, r
    Bv3 = np.stack([
        np.stack([np.broadcast_to(bv[(l, nm)], (P, D)) for nm in ('w', 'c', 'r')])
        for l in range(L)]).astype(np.float32)

    Wo = np.asarray(inputs['out_w'], np.float32).astype(BF)
    Bo = np.asarray(inputs['out_b'], np.float32)[..., None]
    sig = 1.0 / (1.0 + np.exp(-np.asarray(inputs['skip'], np.float32)))
    sig_rep = np.broadcast_to(sig.reshape(1, L * 2),
                              (P, L * 2)).astype(np.float32).copy()
    projw = np.asarray(inputs['proj_w'], np.float32).astype(BF)
    projbT = np.asarray(inputs['proj_b'], np.float32)[..., None]
    clsw = np.asarray(inputs['cls_w'], np.float32).astype(BF)
    clsb_rep = np.broadcast_to(np.asarray(inputs['cls_b'], np.float32),
                               (P, OUT)).copy()

    TPw = TW + TC
    iotar_rep = np.broadcast_to(np.tile(np.arange(P, dtype=np.float32), TPw),
                                (P, TPw * P)).astype(BF).copy()
    iotac_rep = np.broadcast_to(np.arange(P, dtype=np.float32)[:, None],
                                (P, TPw * P)).astype(BF).copy()
    ident = np.eye(P, dtype=BF)

    shared = dict(Wkv3=Wkv3, Wq_aug=Wq_aug, Bq_rep=Bq_rep, Bv3=Bv3,
                  Wo=Wo, Bo=Bo, sig_rep=sig_rep, projw=projw, projbT=projbT,
                  clsw=clsw, clsb_rep=clsb_rep,
                  iotar_rep=iotar_rep, iotac_rep=iotac_rep, ident=ident)

    in_maps = []
    for c in range(C):
        m = dict(shared)
        m['xT_a'] = np.ascontiguousarray(x_a[c * nl:(c + 1) * nl].T).astype(BF)
        m['xT_p'] = np.ascontiguousarray(x_p[c * nl:(c + 1) * nl].T).astype(BF)
        m['idxP'] = idxP[c]
        m['drlP'] = drlP[c]
        m['drlrP'] = drlrP[c]
        m['idxR'] = idxR[c]
        m['drlR'] = drlR[c]
        m['drlrR'] = drlrR[c]
        in_maps.append(m)
    return in_maps, TW, TC, TR, nb


# ---------------------------------------------------------------- device program
def build_program(nl, nb, TW, TC, TR):
    nc = bacc.Bacc()
    dp = nc.declare_dram_parameter
    TP = TW + TC
    NPAD = nb * P

    xT_a = dp('xT_a', [D, nl], BF16, isOutput=False)
    xT_p = dp('xT_p', [D, nl], BF16, isOutput=False)
    idxP_d = dp('idxP', [P, nb * TP], I32, isOutput=False)
    drlP_d = dp('drlP', [P, nb * TP], BF16, isOutput=False)
    drlrP_d = dp('drlrP', [nb, TP * P], BF16, isOutput=False)
    idxR_d = dp('idxR', [P, nb * TR], I32, isOutput=False)
    drlR_d = dp('drlR', [P, nb * TR], BF16, isOutput=False)
    drlrR_d = dp('drlrR', [nb, TR * P], BF16, isOutput=False)
    Wkv3_d = dp('Wkv3', [L, D, 3 * 256], BF16, isOutput=False)
    Wq_d = dp('Wq_aug', [L, 2, D, QW], BF16, isOutput=False)
    Bq_d = dp('Bq_rep', [L, 2, P, QW], F32, isOutput=False)
    Bv3_d = dp('Bv3', [L, 3, P, D], F32, isOutput=False)
    Wo_d = dp('Wo', [L, 2, D, D], BF16, isOutput=False)
    Bo_d = dp('Bo', [L, 2, D, 1], F32, isOutput=False)
    sig_d = dp('sig_rep', [P, L * 2], F32, isOutput=False)
    projw_d = dp('projw', [2, D, D], BF16, isOutput=False)
    projbT_d = dp('projbT', [2, D, 1], F32, isOutput=False)
    clsw_d = dp('clsw', [D, OUT], BF16, isOutput=False)
    clsb_d = dp('clsb_rep', [P, OUT], F32, isOutput=False)
    iota_d = dp('iotar_rep', [P, TP * P], BF16, isOutput=False)
    iotac_d = dp('iotac_rep', [P, TP * P], BF16, isOutput=False)
    ident_d = dp('ident', [P, P], BF16, isOutput=False)
    logits = dp('logits', [nl, OUT], F32, isOutput=True)

    chunks128 = [(i * P, min(P, nl - i * P)) for i in range(_ceil_div(nl, P))]
    WCH = 512
    chunksw = [(i * WCH, min(WCH, nl - i * WCH)) for i in range(_ceil_div(nl, WCH))]

    def seg_chunks(segs):
        out = []
        for (s, ln, r) in segs:
            i = 0
            while i < ln:
                k = min(3, ln - i)
                out.append((s + i, k, r))
                i += k
        return out

    pap_chunks = seg_chunks([(0, TC, 0), (TC, TW, 1)])
    rev_chunks = seg_chunks([(0, TR, 0)])

    with tile.TileContext(nc) as tc:
        with (
            tc.tile_pool(name='const', bufs=1) as cpool,
            tc.tile_pool(name='resid', bufs=1) as rpool,
            tc.tile_pool(name='wpool', bufs=2) as wpool,
            tc.tile_pool(name='sb', bufs=2) as sb,
            tc.tile_pool(name='edg', bufs=2) as edg,
            tc.tile_pool(name='ps_mm', bufs=2, space='PSUM') as ps_mm,
            tc.tile_pool(name='ps_kv', bufs=2, space='PSUM') as ps_kv,
            tc.tile_pool(name='ps_tp', bufs=2, space='PSUM') as ps_tp,
            tc.tile_pool(name='ps_agg', bufs=2, space='PSUM') as ps_agg,
            tc.tile_pool(name='dram', bufs=1, space='DRAM') as dram,
        ):
            # ---- persistent DRAM scratch
            hT_a = dram.tile([D, nl], F32, tag='hT_a')
            hT_p = dram.tile([D, nl], F32, tag='hT_p')
            hT = {0: hT_a, 1: hT_p}
            hrow_A = dram.tile([nl, D], BF16, tag='hrow_A')
            hrow_P = dram.tile([nl, D], BF16, tag='hrow_P')
            hrow = {0: hrow_A, 1: hrow_P}
            sh = 'Shared' if USE_SHARED else 'Local'
            h_tabA = [dram.tile([nl * C, D], BF16, tag=f'h_tabA{l}',
                                name=f'h_tabA{l}', addr_space=sh)
                      for l in range(L)]
            h_tabP = [dram.tile([nl * C, D], BF16, tag=f'h_tabP{l}',
                                name=f'h_tabP{l}', addr_space=sh)
                      for l in range(L)]

            def trigger_ag(l, t):
                nc.gpsimd.collective_compute(
                    'AllGather', mybir.AluOpType.bypass,
                    ins=[hrow[t][:].opt()],
                    outs=[(h_tabA if t == 0 else h_tabP)[l][:].opt()],
                    replica_groups=[list(range(C))])

            # ---- SBUF-resident tables & constants
            idxP_t = rpool.tile([P, nb * TP], I32)
            nc.sync.dma_start(out=idxP_t[:], in_=idxP_d[:, :])
            drlP_t = rpool.tile([P, nb * TP], BF16)
            nc.sync.dma_start(out=drlP_t[:], in_=drlP_d[:, :])
            idxR_t = rpool.tile([P, nb * TR], I32)
            nc.sync.dma_start(out=idxR_t[:], in_=idxR_d[:, :])
            drlR_t = rpool.tile([P, nb * TR], BF16)
            nc.sync.dma_start(out=drlR_t[:], in_=drlR_d[:, :])
            qtab_a = rpool.tile([P, nb * QW], BF16)
            qtab_p = rpool.tile([P, nb * QW], BF16)
            qtab = {0: qtab_a, 1: qtab_p}
            aggT_a = rpool.tile([P, NPAD], BF16)
            aggT_p = rpool.tile([P, NPAD], BF16)
            aggT = {0: aggT_a, 1: aggT_p}
            nc.vector.memset(qtab_a[:], 0.0)
            nc.vector.memset(qtab_p[:], 0.0)

            iotar_rep = cpool.tile([P, TP * P], BF16)
            nc.sync.dma_start(out=iotar_rep[:], in_=iota_d[:, :])
            iotac_rep = cpool.tile([P, TP * P], BF16)
            nc.sync.dma_start(out=iotac_rep[:], in_=iotac_d[:, :])
            ident = cpool.tile([P, P], BF16)
            nc.sync.dma_start(out=ident[:], in_=ident_d[:, :])
            sig_t = cpool.tile([P, L * 2], F32)
            nc.sync.dma_start(out=sig_t[:], in_=sig_d[:, :])
            clsb_t = cpool.tile([P, OUT], F32)
            nc.sync.dma_start(out=clsb_t[:], in_=clsb_d[:, :])
            cw = cpool.tile([D, OUT], BF16)
            nc.sync.dma_start(out=cw[:], in_=clsw_d[:, :])

            def hrow_store(hbsb, t, o, m):
                """hbsb [D, m] bf16 (feature-major) -> hrow_in rows t*nl+o.."""
                nsub = _ceil_div(m, P)
                hrsb = sb.tile([P, WCH], BF16, tag='hrsb')
                for j in range(nsub):
                    mm = min(P, m - j * P)
                    tps = ps_tp.tile([P, P], BF16, tag='tp')
                    nc.tensor.transpose(out=tps[:mm, :],
                                        in_=hbsb[:, j * P:j * P + mm],
                                        identity=ident[:])
                    nc.scalar.activation(out=hrsb[:mm, j * P:(j + 1) * P],
                                         in_=tps[:mm, :], func=AF.Identity)
                for j in range(nsub):
                    mm = min(P, m - j * P)
                    nc.sync.dma_start(
                        out=hrow[t][o + j * P:o + j * P + mm, :],
                        in_=hrsb[:mm, j * P:(j + 1) * P])

            # ---- initial projection: hT[t] (f32) and hrow (bf16)
            for t, xT in ((1, xT_p), (0, xT_a)):
                pw = wpool.tile([D, D], BF16, tag='pw')
                nc.sync.dma_start(out=pw[:], in_=projw_d[t, :, :])
                pb = wpool.tile([D, 1], F32, tag='pb')
                nc.sync.dma_start(out=pb[:], in_=projbT_d[t, :, :])
                for (o, m) in chunksw:
                    xc = sb.tile([D, WCH], BF16, tag='xc')
                    nc.sync.dma_start(out=xc[:, :m], in_=xT[:, o:o + m])
                    ps = ps_mm.tile([P, WCH], F32, tag='mm')
                    nc.tensor.matmul(ps[:, :m], lhsT=pw[:], rhs=xc[:, :m],
                                     start=True, stop=True)
                    hc = sb.tile([D, WCH], F32, tag='hc')
                    nc.scalar.activation(out=hc[:, :m], in_=ps[:, :m],
                                         func=AF.Identity, bias=pb[:])
                    nc.sync.dma_start(out=hT[t][:, o:o + m], in_=hc[:, :m])
                    hb = sb.tile([D, WCH], BF16, tag='hb')
                    nc.vector.tensor_copy(out=hb[:, :m], in_=hc[:, :m])
                    hrow_store(hb, t, o, m)
                trigger_ag(0, t)

            # ================================================= layers
            for l in range(L):
                last = (l == L - 1)

                # ---- node phase: q tables (papers; authors only if rev runs)
                for t in ((1,) if last else (1, 0)):
                    wq = wpool.tile([D, QW], BF16, tag='wq')
                    nc.sync.dma_start(out=wq[:], in_=Wq_d[l, t, :, :])
                    bqt = wpool.tile([P, QW], F32, tag='bqt')
                    nc.sync.dma_start(out=bqt[:], in_=Bq_d[l, t, :, :])
                    for bi, (o, m) in enumerate(chunks128):
                        hcc0 = sb.tile([D, P], F32, tag='hcc0')
                        nc.sync.dma_start(out=hcc0[:, :m], in_=hT[t][:, o:o + m])
                        hcc = sb.tile([D, P], BF16, tag='hcc')
                        nc.vector.tensor_copy(out=hcc[:, :m], in_=hcc0[:, :m])
                        ps = ps_mm.tile([P, WCH], F32, tag='mm')
                        nc.tensor.matmul(ps[:m, :QW], lhsT=hcc[:, :m], rhs=wq[:],
                                         start=True, stop=True)
                        nc.vector.tensor_add(
                            out=qtab[t][:m, bi * QW:(bi + 1) * QW],
                            in0=ps[:m, :QW], in1=bqt[:m, :])

                wkv = wpool.tile([D, 3 * 256], BF16, tag='wkv')
                nc.sync.dma_start(out=wkv[:], in_=Wkv3_d[l, :, :])
                bvw_t = wpool.tile([P, D], F32, tag='bvw')
                nc.sync.dma_start(out=bvw_t[:], in_=Bv3_d[l, 0, :, :])
                bvc_t = wpool.tile([P, D], F32, tag='bvc')
                nc.sync.dma_start(out=bvc_t[:], in_=Bv3_d[l, 1, :, :])
                bvr_t = wpool.tile([P, D], F32, tag='bvr')
                nc.sync.dma_start(out=bvr_t[:], in_=Bv3_d[l, 2, :, :])

                # ---- edge pass helper
                def edge_pass(t, T, idx_t, drl_t, drlr_dram, chunks, wslices,
                              tabs, zsplit, co=None):
                    qt = qtab[t]
                    at = aggT[t]
                    for b in range(nb):
                        c0 = b * T
                        Hg = edg.tile([P, TP * D], BF16, tag='Hg', bufs=3)
                        for g in range(T):
                            nc.gpsimd.indirect_dma_start(
                                out=Hg[:, g * D:(g + 1) * D], out_offset=None,
                                in_=tabs[g][:],
                                in_offset=bass.IndirectOffsetOnAxis(
                                    ap=idx_t[:, c0 + g:c0 + g + 1], axis=0))
                        HgT = edg.tile([P, TP * D], BF16, tag='HgT')
                        for g in range(T):
                            tps = ps_tp.tile([P, P], BF16, tag='tp')
                            nc.tensor.transpose(out=tps[:],
                                                in_=Hg[:, g * D:(g + 1) * D],
                                                identity=ident[:])
                            nc.scalar.activation(
                                out=HgT[:, g * D:(g + 1) * D], in_=tps[:],
                                func=AF.Identity)
                        kvsb = edg.tile([P, TP * 256], BF16, tag='kvsb')
                        for g in range(T):
                            kvp = ps_kv.tile([P, 256], F32, tag='kv')
                            nc.tensor.matmul(kvp[:],
                                             lhsT=HgT[:, g * D:(g + 1) * D],
                                             rhs=wkv[:, wslices[g]],
                                             start=True, stop=True)
                            nc.scalar.activation(
                                out=kvsb[:, g * 256:(g + 1) * 256], in_=kvp[:],
                                func=AF.Identity)
                        onehot = edg.tile([P, TP * P], BF16, tag='onehot')
                        nc.vector.tensor_tensor(
                            out=onehot[:, :T * P].rearrange(
                                'p (t q) -> p t q', q=P),
                            in0=drl_t[:, c0:c0 + T][:, :, None]
                                .to_broadcast([P, T, P]),
                            in1=iotar_rep[:, :T * P].rearrange(
                                'p (t q) -> p t q', q=P),
                            op=mybir.AluOpType.is_equal)
                        drlr = edg.tile([P, TP * P], BF16, tag='drlr')
                        nc.sync.dma_start(
                            out=drlr[:, :T * P],
                            in_=drlr_dram[b:b + 1, :].to_broadcast([P, T * P]))
                        onehotT = edg.tile([P, TP * P], BF16, tag='onehotT')
                        nc.vector.tensor_tensor(
                            out=onehotT[:, :T * P],
                            in0=iotac_rep[:, :T * P],
                            in1=drlr[:, :T * P],
                            op=mybir.AluOpType.is_equal)
                        sc = edg.tile([P, TP * H], F32, tag='sc')
                        for (g0, k, r) in chunks:
                            qe3 = ps_mm.tile([P, WCH], F32, tag='mm')
                            for i in range(k):
                                nc.tensor.matmul(
                                    qe3[:, i * QW:(i + 1) * QW],
                                    lhsT=onehotT[:, (g0 + i) * P:(g0 + i + 1) * P],
                                    rhs=qt[:, b * QW:(b + 1) * QW],
                                    start=True, stop=True)
                            qk3 = edg.tile([P, 3 * D], BF16, tag='qk')
                            nc.vector.tensor_mul(
                                out=qk3[:, :k * D].rearrange(
                                    'p (t w) -> p t w', w=D),
                                in0=qe3[:, :k * QW].rearrange(
                                    'p (t w) -> p t w', w=QW)[:, :, 0:D],
                                in1=kvsb[:, g0 * 256:(g0 + k) * 256].rearrange(
                                    'p (t w) -> p t w', w=256)[:, :, 0:D])
                            scr = edg.tile([P, 3 * H], F32, tag='scr')
                            nc.vector.reduce_sum(
                                out=scr[:, :k * H].rearrange(
                                    'p (t h) -> p t h', h=H),
                                in_=qk3[:, :k * D].rearrange(
                                    'p (t h q) -> p t h q', h=H, q=DH),
                                axis=mybir.AxisListType.X)
                            nc.vector.tensor_add(
                                out=sc[:, g0 * H:(g0 + k) * H].rearrange(
                                    'p (t h) -> p t h', h=H),
                                in0=scr[:, :k * H].rearrange(
                                    'p (t h) -> p t h', h=H),
                                in1=qe3[:, :k * QW].rearrange(
                                    'p (t w) -> p t w', w=QW)
                                    [:, :, D + r * H:D + (r + 1) * H])
                        scexp = edg.tile([P, TP * H], BF16, tag='scexp')
                        nc.scalar.activation(out=scexp[:, :T * H],
                                             in_=sc[:, :T * H], func=AF.Exp)
                        work = edg.tile([P, TP * 132], BF16, tag='work')
                        wv = work[:, :T * 132].rearrange('p (t w) -> p t w', w=132)
                        nc.vector.tensor_copy(
                            out=wv[:, :, 128:132],
                            in_=scexp[:, :T * H].rearrange('p (t h) -> p t h', h=H))
                        nc.vector.tensor_mul(
                            out=wv[:, :, 0:128].rearrange(
                                'p t (h q) -> p t h q', q=DH),
                            in0=kvsb[:, :T * 256].rearrange(
                                'p (t w) -> p t w', w=256)[:, :, 128:256]
                                .rearrange('p t (h q) -> p t h q', q=DH),
                            in1=scexp[:, :T * H].rearrange('p (t h) -> p t h', h=H)
                                [:, :, :, None].to_broadcast([P, T, H, DH]))
                        aggp = ps_agg.tile([P, 136], F32, tag='agg')
                        for g in range(T):
                            nc.tensor.matmul(aggp[:, :132],
                                             lhsT=onehot[:, g * P:(g + 1) * P],
                                             rhs=work[:, g * 132:(g + 1) * 132],
                                             start=(g == 0), stop=(g == T - 1))
                        zr = edg.tile([P, H], F32, tag='zr')
                        nc.vector.tensor_scalar_add(out=zr[:],
                                                    in0=aggp[:, 128:132],
                                                    scalar1=1e-16)
                        zrec = edg.tile([P, H], F32, tag='zrec')
                        nc.vector.reciprocal(out=zrec[:], in_=zr[:])
                        araw = edg.tile([P, D], F32, tag='araw')
                        if zsplit is not None:
                            bv0_t, bv1_t = zsplit
                            for g in range(TC):
                                nc.tensor.matmul(
                                    aggp[:, 132:136],
                                    lhsT=onehot[:, g * P:(g + 1) * P],
                                    rhs=scexp[:, g * H:(g + 1) * H],
                                    start=(g == 0), stop=(g == TC - 1))
                            zw_sb = edg.tile([P, H], F32, tag='zw_sb')
                            nc.vector.tensor_copy(out=zw_sb[:], in_=aggp[:, 132:136])
                            zc_sb = edg.tile([P, H], F32, tag='zc_sb')
                            nc.vector.tensor_sub(out=zc_sb[:],
                                                 in0=aggp[:, 128:132],
                                                 in1=zw_sb[:])
                            f1 = edg.tile([P, D], F32, tag='f1')
                            nc.vector.tensor_mul(
                                out=f1[:].rearrange('p (h q) -> p h q', q=DH),
                                in0=zw_sb[:, :, None].to_broadcast([P, H, DH]),
                                in1=bv0_t[:].rearrange('p (h q) -> p h q', q=DH))
                            f2 = edg.tile([P, D], F32, tag='f2')
                            nc.vector.tensor_mul(
                                out=f2[:].rearrange('p (h q) -> p h q', q=DH),
                                in0=zc_sb[:, :, None].to_broadcast([P, H, DH]),
                                in1=bv1_t[:].rearrange('p (h q) -> p h q', q=DH))
                            nc.vector.tensor_add(out=f1[:], in0=f1[:], in1=f2[:])
                            nc.vector.tensor_add(out=araw[:], in0=aggp[:, 0:128],
                                                 in1=f1[:])
                        else:
                            f1 = edg.tile([P, D], F32, tag='f1')
                            nc.vector.tensor_mul(
                                out=f1[:].rearrange('p (h q) -> p h q', q=DH),
                                in0=aggp[:, 128:132][:, :, None]
                                    .to_broadcast([P, H, DH]),
                                in1=bvr_t[:].rearrange('p (h q) -> p h q', q=DH))
                            nc.vector.tensor_add(out=araw[:], in0=aggp[:, 0:128],
                                                 in1=f1[:])
                        aggd = edg.tile([P, D], BF16, tag='aggd')
                        nc.vector.tensor_mul(
                            out=aggd[:].rearrange('p (h q) -> p h q', q=DH),
                            in0=araw[:].rearrange('p (h q) -> p h q', q=DH),
                            in1=zrec[:, :, None].to_broadcast([P, H, DH]))
                        tpsa = ps_tp.tile([P, P], BF16, tag='tp')
                        nc.tensor.transpose(out=tpsa[:], in_=aggd[:],
                                            identity=ident[:])
                        nc.scalar.activation(out=at[:, b * P:(b + 1) * P],
                                             in_=tpsa[:], func=AF.Identity)
                        if co is not None and b % 4 == 3 and co:
                            co.pop(0)()
                    if co is not None:
                        while co:
                            co.pop(0)()

                def out_phase(t, with_cls, defer=False):
                    wo = wpool.tile([D, D], BF16, tag='wo')
                    nc.sync.dma_start(out=wo[:], in_=Wo_d[l, t, :, :])
                    bo = wpool.tile([D, 1], F32, tag='bo')
                    nc.sync.dma_start(out=bo[:], in_=Bo_d[l, t, :, :])

                    def emit_chunk(o, m):
                        gag = sb.tile([D, WCH], BF16, tag='gag')
                        nc.scalar.activation(out=gag[:, :m],
                                             in_=aggT[t][:, o:o + m],
                                             func=AF.Gelu)
                        ps = ps_mm.tile([P, WCH], F32, tag='mm')
                        nc.tensor.matmul(ps[:, :m], lhsT=wo[:], rhs=gag[:, :m],
                                         start=True, stop=True)
                        ob = sb.tile([D, WCH], F32, tag='tA')
                        nc.scalar.activation(out=ob[:, :m], in_=ps[:, :m],
                                             func=AF.Identity, bias=bo[:])
                        hld = sb.tile([D, WCH], F32, tag='hld')
                        nc.sync.dma_start(out=hld[:, :m], in_=hT[t][:, o:o + m])
                        dif = sb.tile([D, WCH], F32, tag='tB')
                        nc.vector.tensor_sub(out=dif[:, :m], in0=ob[:, :m],
                                             in1=hld[:, :m])
                        sd = sb.tile([D, WCH], F32, tag='tA')
                        nc.vector.tensor_scalar_mul(
                            out=sd[:, :m], in0=dif[:, :m],
                            scalar1=sig_t[:, l * 2 + t:l * 2 + t + 1])
                        hpre = sb.tile([D, WCH], F32, tag='tC')
                        nc.vector.tensor_add(out=hpre[:, :m], in0=sd[:, :m],
                                             in1=hld[:, :m])
                        neg = sb.tile([D, WCH], F32, tag='tB')
                        nc.vector.tensor_scalar_min(out=neg[:, :m],
                                                    in0=hpre[:, :m], scalar1=0.0)
                        ex = sb.tile([D, WCH], F32, tag='tA')
                        nc.scalar.activation(out=ex[:, :m], in_=neg[:, :m],
                                             func=AF.Exp)
                        rl = sb.tile([D, WCH], F32, tag='tB')
                        nc.vector.tensor_scalar_max(out=rl[:, :m],
                                                    in0=hpre[:, :m], scalar1=0.0)
                        er = sb.tile([D, WCH], F32, tag='tA')
                        nc.vector.tensor_add(out=er[:, :m], in0=ex[:, :m],
                                             in1=rl[:, :m])
                        hnew = sb.tile([D, WCH], F32, tag='tC')
                        nc.vector.tensor_scalar_add(out=hnew[:, :m],
                                                    in0=er[:, :m], scalar1=-1.0)
                        if not (last and t == 1):
                            nc.sync.dma_start(out=hT[t][:, o:o + m],
                                              in_=hnew[:, :m])
                        hb = sb.tile([D, WCH], BF16, tag='hb')
                        nc.vector.tensor_copy(out=hb[:, :m], in_=hnew[:, :m])
                        if not last:
                            hrow_store(hb, t, o, m)
                        if with_cls:
                            nsub = _ceil_div(m, P)
                            for j in range(nsub):
                                mm = min(P, m - j * P)
                                ps2 = ps_kv.tile([P, 256], F32, tag='kv')
                                nc.tensor.matmul(ps2[:mm, :OUT],
                                                 lhsT=hb[:, j * P:j * P + mm],
                                                 rhs=cw[:],
                                                 start=True, stop=True)
                                lg = sb.tile([P, OUT], F32, tag='lg')
                                nc.vector.tensor_add(out=lg[:mm, :],
                                                     in0=ps2[:mm, :OUT],
                                                     in1=clsb_t[:mm, :])
                                nc.sync.dma_start(
                                    out=logits[o + j * P:o + j * P + mm, :],
                                    in_=lg[:mm, :])

                    if defer:
                        return [lambda o=o, m=m: emit_chunk(o, m)
                                for (o, m) in chunksw]
                    for (o, m) in chunksw:
                        emit_chunk(o, m)
                    return None

                wsl_pap = [slice(256, 512)] * TC + [slice(0, 256)] * TW
                wsl_rev = [slice(512, 768)] * TR
                tabs_pap = [h_tabP[l]] * TC + [h_tabA[l]] * TW
                tabs_rev = [h_tabP[l]] * TR
                edge_pass(1, TP, idxP_t, drlP_t, drlrP_d, pap_chunks, wsl_pap,
                          tabs_pap, (bvc_t, bvw_t))
                if last:
                    out_phase(1, with_cls=True)
                else:
                    co = out_phase(1, with_cls=False, defer=True)
                    edge_pass(0, TR, idxR_t, drlR_t, drlrR_d, rev_chunks,
                              wsl_rev, tabs_rev, None, co=co)
                    trigger_ag(l + 1, 1)
                    out_phase(0, with_cls=False)
                    trigger_ag(l + 1, 0)

    nc.finalize()
    return nc


# ---------------------------------------------------------------- entry point
_CACHE = {}


def prepare(inputs):
    nn = np.asarray(inputs['x_author']).shape[0]
    nl = nn // C
    in_maps, TW, TC, TR, nb = host_prep(inputs, nl)
    key = (nl, nb, TW, TC, TR)
    if key not in _CACHE:
        _CACHE[key] = build_program(nl, nb, TW, TC, TR)
    return _CACHE[key], in_maps


def kernel(**inputs):
    nc, in_maps = prepare(inputs)
    res = run_bass_kernel_spmd(nc, in_maps, list(range(C)))
    outs = [res.results[c]['logits'] for c in range(C)]
    return np.concatenate(outs, 0)


if __name__ == '__main__':
    pass


# revision 14
# speedup vs baseline: 1.0827x; 1.0827x over previous
"""HGTNet Trainium2 kernel: 8-core SPMD, destination-ownership edge sharding.

v2 strategy:
  - AllGather RAW hidden rows (bf16, Shared scratchpad) once per layer
    (h_tab [200000,128] = 51MB); k/v computed per edge-group on the tensor
    engine from gathered h (relation-pure groups of 128 edge slots).
  - k-bias folded into 8 augmented q-table columns (q_aug = h@Wq_aug);
    v-bias applied via per-relation softmax-mass (z_w/z_c) outer fixup.
    No per-group bias matmuls.
  - onehot from resident drl columns + is_equal; onehotT from a broadcast
    DMA of drl rows + is_equal (DMA engines are otherwise idle).
  - bf16 edge path; fp32 PSUM accumulation; fp32 out-phase mix/ELU chain
    (bf16 there was the dominant error term: 4e-2 -> 4e-3).
  - q tables and aggT SBUF-resident; idx/drl tables SBUF-resident.
  - Layer L-1 skips the rev pass / q_a / out_a entirely (h_a is dead);
    classifier fused into the last out_p pass.
"""
import sys
sys.path.insert(0, '/opt/trn_rl_repo')
import math
import numpy as np
import ml_dtypes

import concourse.bass as bass
import concourse.bacc as bacc
import concourse.mybir as mybir
import concourse.tile as tile
from concourse.bass_utils import run_bass_kernel_spmd

P = 128
D = 128
H = 4
DH = 32
L = 2
OUT = 40
C = 8
QW = D + 2 * H          # augmented q width (136)
SCALE = 1.0 / math.sqrt(DH)

F32 = mybir.dt.float32
BF16 = mybir.dt.bfloat16
I32 = mybir.dt.int32
BF = ml_dtypes.bfloat16

USE_SHARED = True
AF = mybir.ActivationFunctionType


def _ceil_div(a, b):
    return (a + b - 1) // b


# ---------------------------------------------------------------- host prep
def _bucket(src_rows, dst, nl, nb):
    core = dst // nl
    dl = dst % nl
    blk = dl // P
    drl = (dl % P).astype(np.float32)
    key = core.astype(np.int64) * nb + blk
    order = np.argsort(key, kind='stable')
    skey = key[order]
    counts = np.bincount(skey, minlength=C * nb)
    T = int(_ceil_div(int(counts.max()), P))
    offs = np.zeros(C * nb + 1, np.int64)
    offs[1:] = np.cumsum(counts)
    pos = np.arange(len(order), dtype=np.int64) - offs[skey]

    idx_flat = np.zeros((C * nb, T * P), np.int32)
    drl_flat = np.full((C * nb, T * P), -1.0, np.float32)
    idx_flat[skey, pos] = src_rows[order].astype(np.int32)
    drl_flat[skey, pos] = drl[order]
    return (idx_flat.reshape(C, nb, T, P), drl_flat.reshape(C, nb, T, P), T)


def _blockdiag(mats):
    out = np.zeros((D, D), np.float32)
    for h in range(H):
        out[h * DH:(h + 1) * DH, h * DH:(h + 1) * DH] = mats[h]
    return out


def host_prep(inputs, nl):
    nb = _ceil_div(nl, P)
    x_a = np.asarray(inputs['x_author'], np.float32)
    x_p = np.asarray(inputs['x_paper'], np.float32)
    ei_w = np.asarray(inputs['ei_writes']).astype(np.int64)
    ei_r = np.asarray(inputs['ei_rev']).astype(np.int64)
    ei_c = np.asarray(inputs['ei_cites']).astype(np.int64)
    kqv_w = np.asarray(inputs['kqv_w'], np.float32)
    kqv_b = np.asarray(inputs['kqv_b'], np.float32)
    rel_a = np.asarray(inputs['rel_a'], np.float32)
    rel_m = np.asarray(inputs['rel_m'], np.float32)
    p_rel = np.asarray(inputs['p_rel'], np.float32)

    def arow(g):
        return (g // nl) * 2 * nl + (g % nl)

    def prow(g):
        return (g // nl) * 2 * nl + nl + (g % nl)

    idx_w, drl_w, TW = _bucket(arow(ei_w[0]), ei_w[1], nl, nb)
    idx_c, drl_c, TC = _bucket(prow(ei_c[0]), ei_c[1], nl, nb)
    idx_r, drl_r, TR = _bucket(prow(ei_r[0]), ei_r[1], nl, nb)
    TP = TW + TC

    def res_col(a):
        return np.ascontiguousarray(a.transpose(0, 3, 1, 2).reshape(C, P, -1))

    idx_pap = np.concatenate([idx_w, idx_c], axis=2)   # [C, nb, TP, P]
    drl_pap = np.concatenate([drl_w, drl_c], axis=2)
    idxP = res_col(idx_pap)
    idxR = res_col(idx_r)
    drlP = res_col(drl_pap).astype(BF)
    drlR = res_col(drl_r).astype(BF)
    drlrP = np.ascontiguousarray(drl_pap.reshape(C, nb, TP * P).astype(BF))
    drlrR = np.ascontiguousarray(drl_r.reshape(C, nb, TR * P).astype(BF))

    # folded per-relation kv weights (NO bias) packed [D, 768]: w, c, r
    rel_cfg = {'w': (0, 0), 'c': (1, 2), 'r': (1, 1)}
    Wkv3 = np.zeros((L, D, 3 * 256), np.float32)
    bk, bv = {}, {}
    for l in range(L):
        for j, nm in enumerate(('w', 'c', 'r')):
            t, r = rel_cfg[nm]
            Ra = _blockdiag(rel_a[l, r] * p_rel[l, r][:, None, None] * SCALE)
            Rm = _blockdiag(rel_m[l, r])
            Wkv3[l, :, j * 256:j * 256 + 128] = kqv_w[l, 0, t] @ Ra
            Wkv3[l, :, j * 256 + 128:(j + 1) * 256] = kqv_w[l, 2, t] @ Rm
            bk[(l, nm)] = kqv_b[l, 0, t] @ Ra
            bv[(l, nm)] = kqv_b[l, 2, t] @ Rm
    Wkv3 = Wkv3.astype(BF)

    # augmented q: cols 0:128 q; 128:132 q.bk_rel0 per head; 132:136 rel1.
    # dst type 0 (authors): rel0 = 'r'. dst type 1 (papers): rel0='w', rel1='c'.
    Wq_aug = np.zeros((L, 2, D, QW), np.float32)
    Bq_aug = np.zeros((L, 2, 1, QW), np.float32)
    for l in range(L):
        for t in (0, 1):
            Wq = kqv_w[l, 1, t]
            bq = kqv_b[l, 1, t]
            Wq_aug[l, t, :, :D] = Wq
            Bq_aug[l, t, 0, :D] = bq
            rels = ('r',) if t == 0 else ('w', 'c')
            for j, nm in enumerate(rels):
                bkr = bk[(l, nm)]
                for h in range(H):
                    blk = slice(h * DH, (h + 1) * DH)
                    Wq_aug[l, t, :, D + j * H + h] = Wq[:, blk] @ bkr[blk]
                    Bq_aug[l, t, 0, D + j * H + h] = bq[blk] @ bkr[blk]
    Wq_aug = Wq_aug.astype(BF)
    Bq_rep = np.broadcast_to(Bq_aug, (L, 2, P, QW)).astype(np.float32).copy()

    # v-bias fixup tiles [L, 3, P, D] f32 replicated rows: w, c, r
    Bv3 = np.stack([
        np.stack([np.broadcast_to(bv[(l, nm)], (P, D)) for nm in ('w', 'c', 'r')])
        for l in range(L)]).astype(np.float32)

    Wo = np.asarray(inputs['out_w'], np.float32).astype(BF)
    Bo = np.asarray(inputs['out_b'], np.float32)[..., None]
    sig = 1.0 / (1.0 + np.exp(-np.asarray(inputs['skip'], np.float32)))
    sig_rep = np.broadcast_to(sig.reshape(1, L * 2),
                              (P, L * 2)).astype(np.float32).copy()
    projw = np.asarray(inputs['proj_w'], np.float32).astype(BF)
    projbT = np.asarray(inputs['proj_b'], np.float32)[..., None]
    clsw = np.asarray(inputs['cls_w'], np.float32).astype(BF)
    clsb_rep = np.broadcast_to(np.asarray(inputs['cls_b'], np.float32),
                               (P, OUT)).copy()

    iotar_rep = np.broadcast_to(np.tile(np.arange(P, dtype=np.float32), TP),
                                (P, TP * P)).astype(BF).copy()
    iotac_rep = np.broadcast_to(np.arange(P, dtype=np.float32)[:, None],
                                (P, TP * P)).astype(BF).copy()
    ident = np.eye(P, dtype=BF)

    shared = dict(Wkv3=Wkv3, Wq_aug=Wq_aug, Bq_rep=Bq_rep, Bv3=Bv3,
                  Wo=Wo, Bo=Bo, sig_rep=sig_rep, projw=projw, projbT=projbT,
                  clsw=clsw, clsb_rep=clsb_rep,
                  iotar_rep=iotar_rep, iotac_rep=iotac_rep, ident=ident)

    in_maps = []
    for c in range(C):
        m = dict(shared)
        m['xT_a'] = np.ascontiguousarray(x_a[c * nl:(c + 1) * nl].T).astype(BF)
        m['xT_p'] = np.ascontiguousarray(x_p[c * nl:(c + 1) * nl].T).astype(BF)
        m['idxP'] = idxP[c]
        m['drlP'] = drlP[c]
        m['drlrP'] = drlrP[c]
        m['idxR'] = idxR[c]
        m['drlR'] = drlR[c]
        m['drlrR'] = drlrR[c]
        in_maps.append(m)
    return in_maps, TW, TC, TR, nb


# ---------------------------------------------------------------- device program
def build_program(nl, nb, TW, TC, TR):
    nc = bacc.Bacc()
    dp = nc.declare_dram_parameter
    TP = TW + TC
    NPAD = nb * P

    xT_a = dp('xT_a', [D, nl], BF16, isOutput=False)
    xT_p = dp('xT_p', [D, nl], BF16, isOutput=False)
    idxP_d = dp('idxP', [P, nb * TP], I32, isOutput=False)
    drlP_d = dp('drlP', [P, nb * TP], BF16, isOutput=False)
    drlrP_d = dp('drlrP', [nb, TP * P], BF16, isOutput=False)
    idxR_d = dp('idxR', [P, nb * TR], I32, isOutput=False)
    drlR_d = dp('drlR', [P, nb * TR], BF16, isOutput=False)
    drlrR_d = dp('drlrR', [nb, TR * P], BF16, isOutput=False)
    Wkv3_d = dp('Wkv3', [L, D, 3 * 256], BF16, isOutput=False)
    Wq_d = dp('Wq_aug', [L, 2, D, QW], BF16, isOutput=False)
    Bq_d = dp('Bq_rep', [L, 2, P, QW], F32, isOutput=False)
    Bv3_d = dp('Bv3', [L, 3, P, D], F32, isOutput=False)
    Wo_d = dp('Wo', [L, 2, D, D], BF16, isOutput=False)
    Bo_d = dp('Bo', [L, 2, D, 1], F32, isOutput=False)
    sig_d = dp('sig_rep', [P, L * 2], F32, isOutput=False)
    projw_d = dp('projw', [2, D, D], BF16, isOutput=False)
    projbT_d = dp('projbT', [2, D, 1], F32, isOutput=False)
    clsw_d = dp('clsw', [D, OUT], BF16, isOutput=False)
    clsb_d = dp('clsb_rep', [P, OUT], F32, isOutput=False)
    iota_d = dp('iotar_rep', [P, TP * P], BF16, isOutput=False)
    iotac_d = dp('iotac_rep', [P, TP * P], BF16, isOutput=False)
    ident_d = dp('ident', [P, P], BF16, isOutput=False)
    logits = dp('logits', [nl, OUT], F32, isOutput=True)

    chunks128 = [(i * P, min(P, nl - i * P)) for i in range(_ceil_div(nl, P))]
    WCH = 512
    chunksw = [(i * WCH, min(WCH, nl - i * WCH)) for i in range(_ceil_div(nl, WCH))]

    def seg_chunks(segs):
        out = []
        for (s, ln, r) in segs:
            i = 0
            while i < ln:
                k = min(3, ln - i)
                out.append((s + i, k, r))
                i += k
        return out

    pap_chunks = seg_chunks([(0, TW, 0), (TW, TC, 1)])
    rev_chunks = seg_chunks([(0, TR, 0)])

    with tile.TileContext(nc) as tc:
        with (
            tc.tile_pool(name='const', bufs=1) as cpool,
            tc.tile_pool(name='resid', bufs=1) as rpool,
            tc.tile_pool(name='wpool', bufs=2) as wpool,
            tc.tile_pool(name='sb', bufs=2) as sb,
            tc.tile_pool(name='edg', bufs=2) as edg,
            tc.tile_pool(name='ps_mm', bufs=2, space='PSUM') as ps_mm,
            tc.tile_pool(name='ps_kv', bufs=2, space='PSUM') as ps_kv,
            tc.tile_pool(name='ps_tp', bufs=2, space='PSUM') as ps_tp,
            tc.tile_pool(name='ps_agg', bufs=2, space='PSUM') as ps_agg,
            tc.tile_pool(name='dram', bufs=1, space='DRAM') as dram,
        ):
            # ---- persistent DRAM scratch
            hT_a = dram.tile([D, nl], F32, tag='hT_a')
            hT_p = dram.tile([D, nl], F32, tag='hT_p')
            hT = {0: hT_a, 1: hT_p}
            hrow_in = dram.tile([2 * nl, D], BF16, tag='hrow_in')
            h_tabs = [
                dram.tile([2 * nl * C, D], BF16, tag=f'h_tab{l}',
                          name=f'h_tab{l}',
                          addr_space='Shared' if USE_SHARED else 'Local')
                for l in range(L)
            ]

            # ---- SBUF-resident tables & constants
            idxP_t = rpool.tile([P, nb * TP], I32)
            nc.sync.dma_start(out=idxP_t[:], in_=idxP_d[:, :])
            drlP_t = rpool.tile([P, nb * TP], BF16)
            nc.sync.dma_start(out=drlP_t[:], in_=drlP_d[:, :])
            idxR_t = rpool.tile([P, nb * TR], I32)
            nc.sync.dma_start(out=idxR_t[:], in_=idxR_d[:, :])
            drlR_t = rpool.tile([P, nb * TR], BF16)
            nc.sync.dma_start(out=drlR_t[:], in_=drlR_d[:, :])
            qtab_a = rpool.tile([P, nb * QW], BF16)
            qtab_p = rpool.tile([P, nb * QW], BF16)
            qtab = {0: qtab_a, 1: qtab_p}
            aggT_a = rpool.tile([P, NPAD], BF16)
            aggT_p = rpool.tile([P, NPAD], BF16)
            aggT = {0: aggT_a, 1: aggT_p}
            nc.vector.memset(qtab_a[:], 0.0)
            nc.vector.memset(qtab_p[:], 0.0)

            iotar_rep = cpool.tile([P, TP * P], BF16)
            nc.sync.dma_start(out=iotar_rep[:], in_=iota_d[:, :])
            iotac_rep = cpool.tile([P, TP * P], BF16)
            nc.sync.dma_start(out=iotac_rep[:], in_=iotac_d[:, :])
            ident = cpool.tile([P, P], BF16)
            nc.sync.dma_start(out=ident[:], in_=ident_d[:, :])
            sig_t = cpool.tile([P, L * 2], F32)
            nc.sync.dma_start(out=sig_t[:], in_=sig_d[:, :])
            clsb_t = cpool.tile([P, OUT], F32)
            nc.sync.dma_start(out=clsb_t[:], in_=clsb_d[:, :])
            cw = cpool.tile([D, OUT], BF16)
            nc.sync.dma_start(out=cw[:], in_=clsw_d[:, :])

            def hrow_store(hbsb, t, o, m):
                """hbsb [D, m] bf16 (feature-major) -> hrow_in rows t*nl+o.."""
                nsub = _ceil_div(m, P)
                hrsb = sb.tile([P, WCH], BF16, tag='hrsb')
                for j in range(nsub):
                    mm = min(P, m - j * P)
                    tps = ps_tp.tile([P, P], BF16, tag='tp')
                    nc.tensor.transpose(out=tps[:mm, :],
                                        in_=hbsb[:, j * P:j * P + mm],
                                        identity=ident[:])
                    nc.scalar.activation(out=hrsb[:mm, j * P:(j + 1) * P],
                                         in_=tps[:mm, :], func=AF.Identity)
                for j in range(nsub):
                    mm = min(P, m - j * P)
                    nc.sync.dma_start(
                        out=hrow_in[t * nl + o + j * P:t * nl + o + j * P + mm, :],
                        in_=hrsb[:mm, j * P:(j + 1) * P])

            # ---- initial projection: hT[t] (f32) and hrow (bf16)
            for t, xT in ((0, xT_a), (1, xT_p)):
                pw = wpool.tile([D, D], BF16, tag='pw')
                nc.sync.dma_start(out=pw[:], in_=projw_d[t, :, :])
                pb = wpool.tile([D, 1], F32, tag='pb')
                nc.sync.dma_start(out=pb[:], in_=projbT_d[t, :, :])
                for (o, m) in chunksw:
                    xc = sb.tile([D, WCH], BF16, tag='xc')
                    nc.sync.dma_start(out=xc[:, :m], in_=xT[:, o:o + m])
                    ps = ps_mm.tile([P, WCH], F32, tag='mm')
                    nc.tensor.matmul(ps[:, :m], lhsT=pw[:], rhs=xc[:, :m],
                                     start=True, stop=True)
                    hc = sb.tile([D, WCH], F32, tag='hc')
                    nc.scalar.activation(out=hc[:, :m], in_=ps[:, :m],
                                         func=AF.Identity, bias=pb[:])
                    nc.sync.dma_start(out=hT[t][:, o:o + m], in_=hc[:, :m])
                    hb = sb.tile([D, WCH], BF16, tag='hb')
                    nc.vector.tensor_copy(out=hb[:, :m], in_=hc[:, :m])
                    hrow_store(hb, t, o, m)

            # ================================================= layers
            for l in range(L):
                last = (l == L - 1)
                h_tab = h_tabs[l]
                nc.gpsimd.collective_compute(
                    'AllGather', mybir.AluOpType.bypass,
                    ins=[hrow_in[:].opt()], outs=[h_tab[:].opt()],
                    replica_groups=[list(range(C))])

                # ---- node phase: q tables (papers; authors only if rev runs)
                for t in ((1,) if last else (1, 0)):
                    wq = wpool.tile([D, QW], BF16, tag='wq')
                    nc.sync.dma_start(out=wq[:], in_=Wq_d[l, t, :, :])
                    bqt = wpool.tile([P, QW], F32, tag='bqt')
                    nc.sync.dma_start(out=bqt[:], in_=Bq_d[l, t, :, :])
                    for bi, (o, m) in enumerate(chunks128):
                        hcc0 = sb.tile([D, P], F32, tag='hcc0')
                        nc.sync.dma_start(out=hcc0[:, :m], in_=hT[t][:, o:o + m])
                        hcc = sb.tile([D, P], BF16, tag='hcc')
                        nc.vector.tensor_copy(out=hcc[:, :m], in_=hcc0[:, :m])
                        ps = ps_mm.tile([P, WCH], F32, tag='mm')
                        nc.tensor.matmul(ps[:m, :QW], lhsT=hcc[:, :m], rhs=wq[:],
                                         start=True, stop=True)
                        nc.vector.tensor_add(
                            out=qtab[t][:m, bi * QW:(bi + 1) * QW],
                            in0=ps[:m, :QW], in1=bqt[:m, :])

                wkv = wpool.tile([D, 3 * 256], BF16, tag='wkv')
                nc.sync.dma_start(out=wkv[:], in_=Wkv3_d[l, :, :])
                bvw_t = wpool.tile([P, D], F32, tag='bvw')
                nc.sync.dma_start(out=bvw_t[:], in_=Bv3_d[l, 0, :, :])
                bvc_t = wpool.tile([P, D], F32, tag='bvc')
                nc.sync.dma_start(out=bvc_t[:], in_=Bv3_d[l, 1, :, :])
                bvr_t = wpool.tile([P, D], F32, tag='bvr')
                nc.sync.dma_start(out=bvr_t[:], in_=Bv3_d[l, 2, :, :])

                # ---- edge pass helper
                def edge_pass(t, T, idx_t, drl_t, drlr_dram, chunks, wslices,
                              zsplit):
                    qt = qtab[t]
                    at = aggT[t]
                    for b in range(nb):
                        c0 = b * T
                        Hg = edg.tile([P, TP * D], BF16, tag='Hg', bufs=4)
                        for g in range(T):
                            nc.gpsimd.indirect_dma_start(
                                out=Hg[:, g * D:(g + 1) * D], out_offset=None,
                                in_=h_tab[:],
                                in_offset=bass.IndirectOffsetOnAxis(
                                    ap=idx_t[:, c0 + g:c0 + g + 1], axis=0))
                        HgT = edg.tile([P, TP * D], BF16, tag='HgT')
                        for g in range(T):
                            tps = ps_tp.tile([P, P], BF16, tag='tp')
                            nc.tensor.transpose(out=tps[:],
                                                in_=Hg[:, g * D:(g + 1) * D],
                                                identity=ident[:])
                            if g % 2 == 0:
                                nc.scalar.activation(
                                    out=HgT[:, g * D:(g + 1) * D], in_=tps[:],
                                    func=AF.Identity)
                            else:
                                nc.vector.tensor_copy(
                                    out=HgT[:, g * D:(g + 1) * D], in_=tps[:])
                        kvsb = edg.tile([P, TP * 256], BF16, tag='kvsb')
                        for g in range(T):
                            kvp = ps_kv.tile([P, 256], F32, tag='kv')
                            nc.tensor.matmul(kvp[:],
                                             lhsT=HgT[:, g * D:(g + 1) * D],
                                             rhs=wkv[:, wslices[g]],
                                             start=True, stop=True)
                            nc.scalar.activation(
                                out=kvsb[:, g * 256:(g + 1) * 256], in_=kvp[:],
                                func=AF.Identity)
                        onehot = edg.tile([P, TP * P], BF16, tag='onehot')
                        nc.vector.tensor_tensor(
                            out=onehot[:, :T * P].rearrange(
                                'p (t q) -> p t q', q=P),
                            in0=drl_t[:, c0:c0 + T][:, :, None]
                                .to_broadcast([P, T, P]),
                            in1=iotar_rep[:, :T * P].rearrange(
                                'p (t q) -> p t q', q=P),
                            op=mybir.AluOpType.is_equal)
                        drlr = edg.tile([P, TP * P], BF16, tag='drlr')
                        nc.sync.dma_start(
                            out=drlr[:, :T * P],
                            in_=drlr_dram[b:b + 1, :].to_broadcast([P, T * P]))
                        onehotT = edg.tile([P, TP * P], BF16, tag='onehotT')
                        nc.vector.tensor_tensor(
                            out=onehotT[:, :T * P],
                            in0=iotac_rep[:, :T * P],
                            in1=drlr[:, :T * P],
                            op=mybir.AluOpType.is_equal)
                        sc = edg.tile([P, TP * H], F32, tag='sc')
                        for (g0, k, r) in chunks:
                            qe3 = ps_mm.tile([P, WCH], F32, tag='mm')
                            for i in range(k):
                                nc.tensor.matmul(
                                    qe3[:, i * QW:(i + 1) * QW],
                                    lhsT=onehotT[:, (g0 + i) * P:(g0 + i + 1) * P],
                                    rhs=qt[:, b * QW:(b + 1) * QW],
                                    start=True, stop=True)
                            qk3 = edg.tile([P, 3 * D], BF16, tag='qk')
                            nc.vector.tensor_mul(
                                out=qk3[:, :k * D].rearrange(
                                    'p (t w) -> p t w', w=D),
                                in0=qe3[:, :k * QW].rearrange(
                                    'p (t w) -> p t w', w=QW)[:, :, 0:D],
                                in1=kvsb[:, g0 * 256:(g0 + k) * 256].rearrange(
                                    'p (t w) -> p t w', w=256)[:, :, 0:D])
                            scr = edg.tile([P, 3 * H], F32, tag='scr')
                            nc.vector.reduce_sum(
                                out=scr[:, :k * H].rearrange(
                                    'p (t h) -> p t h', h=H),
                                in_=qk3[:, :k * D].rearrange(
                                    'p (t h q) -> p t h q', h=H, q=DH),
                                axis=mybir.AxisListType.X)
                            nc.vector.tensor_add(
                                out=sc[:, g0 * H:(g0 + k) * H].rearrange(
                                    'p (t h) -> p t h', h=H),
                                in0=scr[:, :k * H].rearrange(
                                    'p (t h) -> p t h', h=H),
                                in1=qe3[:, :k * QW].rearrange(
                                    'p (t w) -> p t w', w=QW)
                                    [:, :, D + r * H:D + (r + 1) * H])
                        scexp = edg.tile([P, TP * H], BF16, tag='scexp')
                        nc.scalar.activation(out=scexp[:, :T * H],
                                             in_=sc[:, :T * H], func=AF.Exp)
                        work = edg.tile([P, TP * 132], BF16, tag='work')
                        wv = work[:, :T * 132].rearrange('p (t w) -> p t w', w=132)
                        nc.vector.tensor_copy(
                            out=wv[:, :, 128:132],
                            in_=scexp[:, :T * H].rearrange('p (t h) -> p t h', h=H))
                        nc.vector.tensor_mul(
                            out=wv[:, :, 0:128].rearrange(
                                'p t (h q) -> p t h q', q=DH),
                            in0=kvsb[:, :T * 256].rearrange(
                                'p (t w) -> p t w', w=256)[:, :, 128:256]
                                .rearrange('p t (h q) -> p t h q', q=DH),
                            in1=scexp[:, :T * H].rearrange('p (t h) -> p t h', h=H)
                                [:, :, :, None].to_broadcast([P, T, H, DH]))
                        aggp = ps_agg.tile([P, 136], F32, tag='agg')
                        for g in range(T):
                            nc.tensor.matmul(aggp[:, :132],
                                             lhsT=onehot[:, g * P:(g + 1) * P],
                                             rhs=work[:, g * 132:(g + 1) * 132],
                                             start=(g == 0), stop=(g == T - 1))
                        zr = edg.tile([P, H], F32, tag='zr')
                        nc.vector.tensor_scalar_add(out=zr[:],
                                                    in0=aggp[:, 128:132],
                                                    scalar1=1e-16)
                        zrec = edg.tile([P, H], F32, tag='zrec')
                        nc.vector.reciprocal(out=zrec[:], in_=zr[:])
                        araw = edg.tile([P, D], F32, tag='araw')
                        if zsplit is not None:
                            bv0_t, bv1_t = zsplit
                            for g in range(TW):
                                nc.tensor.matmul(
                                    aggp[:, 132:136],
                                    lhsT=onehot[:, g * P:(g + 1) * P],
                                    rhs=scexp[:, g * H:(g + 1) * H],
                                    start=(g == 0), stop=(g == TW - 1))
                            zw_sb = edg.tile([P, H], F32, tag='zw_sb')
                            nc.vector.tensor_copy(out=zw_sb[:], in_=aggp[:, 132:136])
                            zc_sb = edg.tile([P, H], F32, tag='zc_sb')
                            nc.vector.tensor_sub(out=zc_sb[:],
                                                 in0=aggp[:, 128:132],
                                                 in1=zw_sb[:])
                            f1 = edg.tile([P, D], F32, tag='f1')
                            nc.vector.tensor_mul(
                                out=f1[:].rearrange('p (h q) -> p h q', q=DH),
                                in0=zw_sb[:, :, None].to_broadcast([P, H, DH]),
                                in1=bv0_t[:].rearrange('p (h q) -> p h q', q=DH))
                            f2 = edg.tile([P, D], F32, tag='f2')
                            nc.vector.tensor_mul(
                                out=f2[:].rearrange('p (h q) -> p h q', q=DH),
                                in0=zc_sb[:, :, None].to_broadcast([P, H, DH]),
                                in1=bv1_t[:].rearrange('p (h q) -> p h q', q=DH))
                            nc.vector.tensor_add(out=f1[:], in0=f1[:], in1=f2[:])
                            nc.vector.tensor_add(out=araw[:], in0=aggp[:, 0:128],
                                                 in1=f1[:])
                        else:
                            f1 = edg.tile([P, D], F32, tag='f1')
                            nc.vector.tensor_mul(
                                out=f1[:].rearrange('p (h q) -> p h q', q=DH),
                                in0=aggp[:, 128:132][:, :, None]
                                    .to_broadcast([P, H, DH]),
                                in1=bvr_t[:].rearrange('p (h q) -> p h q', q=DH))
                            nc.vector.tensor_add(out=araw[:], in0=aggp[:, 0:128],
                                                 in1=f1[:])
                        aggd = edg.tile([P, D], BF16, tag='aggd')
                        nc.vector.tensor_mul(
                            out=aggd[:].rearrange('p (h q) -> p h q', q=DH),
                            in0=araw[:].rearrange('p (h q) -> p h q', q=DH),
                            in1=zrec[:, :, None].to_broadcast([P, H, DH]))
                        tpsa = ps_tp.tile([P, P], BF16, tag='tp')
                        nc.tensor.transpose(out=tpsa[:], in_=aggd[:],
                                            identity=ident[:])
                        nc.scalar.activation(out=at[:, b * P:(b + 1) * P],
                                             in_=tpsa[:], func=AF.Identity)

                def out_phase(t, with_cls):
                    wo = wpool.tile([D, D], BF16, tag='wo')
                    nc.sync.dma_start(out=wo[:], in_=Wo_d[l, t, :, :])
                    bo = wpool.tile([D, 1], F32, tag='bo')
                    nc.sync.dma_start(out=bo[:], in_=Bo_d[l, t, :, :])
                    for (o, m) in chunksw:
                        gag = sb.tile([D, WCH], BF16, tag='gag')
                        nc.scalar.activation(out=gag[:, :m],
                                             in_=aggT[t][:, o:o + m],
                                             func=AF.Gelu)
                        ps = ps_mm.tile([P, WCH], F32, tag='mm')
                        nc.tensor.matmul(ps[:, :m], lhsT=wo[:], rhs=gag[:, :m],
                                         start=True, stop=True)
                        ob = sb.tile([D, WCH], F32, tag='tA')
                        nc.scalar.activation(out=ob[:, :m], in_=ps[:, :m],
                                             func=AF.Identity, bias=bo[:])
                        hld = sb.tile([D, WCH], F32, tag='hld')
                        nc.sync.dma_start(out=hld[:, :m], in_=hT[t][:, o:o + m])
                        dif = sb.tile([D, WCH], F32, tag='tB')
                        nc.vector.tensor_sub(out=dif[:, :m], in0=ob[:, :m],
                                             in1=hld[:, :m])
                        sd = sb.tile([D, WCH], F32, tag='tA')
                        nc.vector.tensor_scalar_mul(
                            out=sd[:, :m], in0=dif[:, :m],
                            scalar1=sig_t[:, l * 2 + t:l * 2 + t + 1])
                        hpre = sb.tile([D, WCH], F32, tag='tC')
                        nc.vector.tensor_add(out=hpre[:, :m], in0=sd[:, :m],
                                             in1=hld[:, :m])
                        neg = sb.tile([D, WCH], F32, tag='tB')
                        nc.vector.tensor_scalar_min(out=neg[:, :m],
                                                    in0=hpre[:, :m], scalar1=0.0)
                        ex = sb.tile([D, WCH], F32, tag='tA')
                        nc.scalar.activation(out=ex[:, :m], in_=neg[:, :m],
                                             func=AF.Exp)
                        rl = sb.tile([D, WCH], F32, tag='tB')
                        nc.vector.tensor_scalar_max(out=rl[:, :m],
                                                    in0=hpre[:, :m], scalar1=0.0)
                        er = sb.tile([D, WCH], F32, tag='tA')
                        nc.vector.tensor_add(out=er[:, :m], in0=ex[:, :m],
                                             in1=rl[:, :m])
                        hnew = sb.tile([D, WCH], F32, tag='tC')
                        nc.vector.tensor_scalar_add(out=hnew[:, :m],
                                                    in0=er[:, :m], scalar1=-1.0)
                        if not (last and t == 1):
                            nc.sync.dma_start(out=hT[t][:, o:o + m],
                                              in_=hnew[:, :m])
                        hb = sb.tile([D, WCH], BF16, tag='hb')
                        nc.vector.tensor_copy(out=hb[:, :m], in_=hnew[:, :m])
                        if not last:
                            hrow_store(hb, t, o, m)
                        if with_cls:
                            nsub = _ceil_div(m, P)
                            for j in range(nsub):
                                mm = min(P, m - j * P)
                                ps2 = ps_kv.tile([P, 256], F32, tag='kv')
                                nc.tensor.matmul(ps2[:mm, :OUT],
                                                 lhsT=hb[:, j * P:j * P + mm],
                                                 rhs=cw[:],
                                                 start=True, stop=True)
                                lg = sb.tile([P, OUT], F32, tag='lg')
                                nc.vector.tensor_add(out=lg[:mm, :],
                                                     in0=ps2[:mm, :OUT],
                                                     in1=clsb_t[:mm, :])
                                nc.sync.dma_start(
                                    out=logits[o + j * P:o + j * P + mm, :],
                                    in_=lg[:mm, :])

                wsl_pap = [slice(0, 256)] * TW + [slice(256, 512)] * TC
                wsl_rev = [slice(512, 768)] * TR
                edge_pass(1, TP, idxP_t, drlP_t, drlrP_d, pap_chunks, wsl_pap,
                          (bvw_t, bvc_t))
                out_phase(1, with_cls=last)
                if not last:
                    edge_pass(0, TR, idxR_t, drlR_t, drlrR_d, rev_chunks,
                              wsl_rev, None)
                    out_phase(0, with_cls=False)

    nc.finalize()
    return nc


# ---------------------------------------------------------------- entry point
_CACHE = {}


def prepare(inputs):
    nn = np.asarray(inputs['x_author']).shape[0]
    nl = nn // C
    in_maps, TW, TC, TR, nb = host_prep(inputs, nl)
    key = (nl, nb, TW, TC, TR)
    if key not in _CACHE:
        _CACHE[key] = build_program(nl, nb, TW, TC, TR)
    return _CACHE[key], in_maps


def kernel(**inputs):
    nc, in_maps = prepare(inputs)
    res = run_bass_kernel_spmd(nc, in_maps, list(range(C)))
    outs = [res.results[c]['logits'] for c in range(C)]
    return np.concatenate(outs, 0)


if __name__ == '__main__':
    pass
